# Initial kernel scaffold
#
"""GPT2 multi-head causal self-attention on 8 trn2 NeuronCores.

Sharding: data-parallel over batch (B=8 -> one batch element per core).
Each core computes its full attention block independently; no collectives.

Per-core dataflow (S=1024, H=768, nh=12, hd=64), all matmuls float32r:
  x [S,H] --PE-transpose--> xT [H,S]
  qkT[c,s] = W_qkv[:, :1536].T @ xT   (Q,K transposed; bias via ACT evac)
  v[s,c]   = x @ W_qkv[:, 1536:]      (natural; bias via DVE evac; ones col
                                       appended per head for the softmax sum)
  per head pair (2 heads share a 128-partition tile, row-packed matmuls):
    scoresT[k,q] = KT.T @ QT chunks (causal: only q >= k-tile start)
    expT = exp(0.125*scoresT) (ACT, no max-sub: |scores/8| < ~3), tri-mask
    att_unT[d,q] (+denom row) = [V_h|1].T @ expT  (PSUM-accumulated over k)
    recip(denom) -> PE-broadcast to 64 rows -> DVE mult = attT[d,q]
  out = attT.T @ W_proj + b_proj  (row-packed pairs, natural layout, DMA out)
"""
import numpy as np

import concourse.bacc as bacc
import concourse.mybir as mybir
import concourse.tile as tile
from concourse.bass_utils import run_bass_kernel_spmd

F32 = mybir.dt.float32
F32R = mybir.dt.float32r
AF = mybir.ActivationFunctionType

B, S, H = 8, 1024, 768
NH, HD = 12, 64
NHT = H // 128          # 6  h-tiles
NST = S // 128          # 8  s-tiles
NQC = S // 512          # 2  q chunks
NPAIR = NH // 2         # 6  head pairs
SCALE = HD ** -0.5

_CACHE = {}


def build():
    nc = bacc.Bacc("TRN2", target_bir_lowering=False, debug=False)

    x_d = nc.declare_dram_parameter("x", [S, H], F32, isOutput=False)
    wqk_d = nc.declare_dram_parameter("wqk", [H, 2 * H], F32R, isOutput=False)
    wv_d = nc.declare_dram_parameter("wv", [H, H], F32R, isOutput=False)
    wp_d = nc.declare_dram_parameter("wp", [H, H], F32R, isOutput=False)
    bqk_d = nc.declare_dram_parameter("bqk", [128, NH], F32, isOutput=False)
    bvb_d = nc.declare_dram_parameter("bvb", [128, H], F32, isOutput=False)
    bpb_d = nc.declare_dram_parameter("bpb", [128, H], F32, isOutput=False)
    mask_d = nc.declare_dram_parameter("mask", [128, 128], F32R, isOutput=False)
    ones65_d = nc.declare_dram_parameter("ones65", [65, 64], F32R, isOutput=False)
    zer64_d = nc.declare_dram_parameter("zer64", [64, 1024], F32R, isOutput=False)
    vones_d = nc.declare_dram_parameter("vones", [128, NST * NH], F32R, isOutput=False)
    ident_d = nc.declare_dram_parameter("ident", [128, 128], F32, isOutput=False)
    out_d = nc.declare_dram_parameter("out", [S, H], F32, isOutput=True)

    with tile.TileContext(nc) as tc:
        with (
            tc.tile_pool(name="perm", bufs=1) as perm,
            tc.tile_pool(name="misc", bufs=2) as misc,
        ):
            # ---- persistent SBUF ----
            wp_sb = perm.tile([128, NHT, H], F32R)
            bqk_sb = perm.tile([128, NH], F32)
            bvb_sb = perm.tile([128, H], F32)
            bpb_sb = perm.tile([128, H], F32)
            mask_sb = perm.tile([128, 128], F32R)
            ones65_sb = perm.tile([65, 64], F32R)
            ident_sb = perm.tile([128, 128], F32)
            rab0 = perm.tile([65, 1024], F32R)
            rab1 = perm.tile([65, 1024], F32R)
            qkT = perm.tile([128, NH, S], F32R)
            v_sb = perm.tile([128, NST, NH, 65], F32R)
            attnT = perm.tile([128, NHT, S], F32R)

            nc.sync.dma_start(ident_sb[:], ident_d[:])
            nc.sync.dma_start(bqk_sb[:], bqk_d[:])
            for ht in range(NHT):
                nc.sync.dma_start(wp_sb[:, ht, :], wp_d[ht * 128:(ht + 1) * 128, :])
            nc.sync.dma_start(bvb_sb[:], bvb_d[:])
            nc.sync.dma_start(bpb_sb[:], bpb_d[:])
            nc.sync.dma_start(mask_sb[:], mask_d[:])
            nc.sync.dma_start(ones65_sb[:], ones65_d[:])
            nc.sync.dma_start(rab0[0:64, :], zer64_d[:])
            nc.sync.dma_start(rab1[0:64, :], zer64_d[:])
            nc.sync.dma_start(
                v_sb[:, :, :, 64],
                vones_d[:].rearrange("p (a b) -> p a b", a=NST))

            # ---- phase 1: transpose x into xT [h, s] ----
            with tc.tile_pool(name="xph", bufs=1) as xph:
                xT = xph.tile([128, NHT, S], F32R)
                with tc.tile_pool(name="pst", bufs=4, space="PSUM") as pst:
                    for rnd in range(2):
                        x_tmp = xph.tile([128, 4, H], F32, tag="x_tmp")
                        nc.sync.dma_start(
                            x_tmp[:],
                            x_d[rnd * 512:(rnd + 1) * 512, :].rearrange(
                                "(st p) c -> p st c", p=128))
                        for st4 in range(4):
                            st = rnd * 4 + st4
                            for ht in range(NHT):
                                ps_t = pst.tile([128, 128], F32, tag="ps_t")
                                nc.tensor.transpose(
                                    ps_t[:], x_tmp[:, st4, ht * 128:(ht + 1) * 128],
                                    ident_sb[:])
                                nc.scalar.copy(xT[:, ht, st * 128:(st + 1) * 128],
                                               ps_t[:])

                # ---- phase 2a: Q,K transposed ----
                with tc.tile_pool(name="wqkp", bufs=1) as wqkp:
                    wqk_sb = wqkp.tile([128, NHT, 2 * H], F32R)
                    for ht in range(NHT):
                        nc.sync.dma_start(wqk_sb[:, ht, :],
                                          wqk_d[ht * 128:(ht + 1) * 128, :])
                    with tc.tile_pool(name="psqk", bufs=3, space="PSUM") as psqk:
                        for t in range(NH):
                            for qc in range(NQC):
                                ps = psqk.tile([128, 512], F32, tag="ps")
                                for ht in range(NHT):
                                    nc.tensor.matmul(
                                        ps[:],
                                        wqk_sb[:, ht, t * 128:(t + 1) * 128],
                                        xT[:, ht, qc * 512:(qc + 1) * 512],
                                        start=(ht == 0), stop=(ht == NHT - 1))
                                nc.scalar.activation(
                                    qkT[:, t, qc * 512:(qc + 1) * 512], ps[:],
                                    AF.Identity, bias=bqk_sb[:, t:t + 1])

                # ---- phase 2b: V natural (+bias), interleaved per head ----
                with tc.tile_pool(name="wvp", bufs=1) as wvp:
                    wv_sb = wvp.tile([128, NHT, H], F32R)
                    for ht in range(NHT):
                        nc.sync.dma_start(wv_sb[:, ht, :],
                                          wv_d[ht * 128:(ht + 1) * 128, :])
                    with tc.tile_pool(name="psv", bufs=3, space="PSUM") as psv:
                        for st in range(NST):
                            for cc in range(2):
                                psv_t = psv.tile([128, 384], F32, tag="psv_t")
                                for ht in range(NHT):
                                    nc.tensor.matmul(
                                        psv_t[:],
                                        xT[:, ht, st * 128:(st + 1) * 128],
                                        wv_sb[:, ht, cc * 384:(cc + 1) * 384],
                                        start=(ht == 0), stop=(ht == NHT - 1))
                                nc.vector.tensor_add(
                                    v_sb[:, st, cc * 6:(cc + 1) * 6, 0:64],
                                    psv_t[:].rearrange("p (g d) -> p g d", g=6),
                                    bvb_sb[:, cc * 384:(cc + 1) * 384].rearrange(
                                        "p (g d) -> p g d", g=6))

            # ---- phase 3: attention per head pair ----
            with (
                tc.tile_pool(name="expp", bufs=1) as expp,
                tc.tile_pool(name="pssc", bufs=2, space="PSUM") as pssc,
                tc.tile_pool(name="psav", bufs=1, space="PSUM") as psav,
                tc.tile_pool(name="psbc", bufs=2, space="PSUM") as psbc,
            ):
                for hp in range(NPAIR):
                    expA = expp.tile([128, NST, S], F32R, tag="expA")
                    expB = expp.tile([128, NST, S], F32R, tag="expB")
                    for kt in range(NST):
                        qc0 = kt // 4
                        d = kt * 128 - qc0 * 512
                        for qc in range(qc0, NQC):
                            cs = d if qc == qc0 else 0
                            qsl = slice(qc * 512 + cs, (qc + 1) * 512)
                            ksl = slice(kt * 128, (kt + 1) * 128)
                            psA = pssc.tile([128, 512], F32, tag="psA")
                            psB = pssc.tile([128, 512], F32, tag="psB")
                            nc.tensor.matmul(
                                psA[:, cs:512], qkT[0:64, 6 + hp, ksl],
                                qkT[0:64, hp, qsl],
                                start=True, stop=True, tile_position=(0, 0))
                            nc.tensor.matmul(
                                psB[:, cs:512], qkT[64:128, 6 + hp, ksl],
                                qkT[64:128, hp, qsl],
                                start=True, stop=True, tile_position=(64, 0))
                            nc.scalar.activation(expA[:, kt, qsl], psA[:, cs:512],
                                                 AF.Exp, scale=SCALE)
                            nc.scalar.activation(expB[:, kt, qsl], psB[:, cs:512],
                                                 AF.Exp, scale=SCALE)
                        dsl = slice(kt * 128, kt * 128 + 128)
                        nc.vector.tensor_mul(expA[:, kt, dsl], expA[:, kt, dsl],
                                             mask_sb[:])
                        nc.vector.tensor_mul(expB[:, kt, dsl], expB[:, kt, dsl],
                                             mask_sb[:])

                    for qc in range(NQC):
                        avA = psav.tile([65, 512], F32, tag="avA")
                        avB = psav.tile([65, 512], F32, tag="avB")
                        kts = list(range(4 * (qc + 1)))
                        for kt in kts:
                            cs = (kt * 128 - qc * 512) if kt // 4 == qc else 0
                            qsl = slice(qc * 512 + cs, (qc + 1) * 512)
                            st, sp = kt == kts[0], kt == kts[-1]
                            nc.tensor.matmul(avA[:, cs:512],
                                             v_sb[:, kt, 2 * hp, :],
                                             expA[:, kt, qsl], start=st, stop=sp)
                            nc.tensor.matmul(avB[:, cs:512],
                                             v_sb[:, kt, 2 * hp + 1, :],
                                             expB[:, kt, qsl], start=st, stop=sp)
                        rab = rab0 if (hp * NQC + qc) % 2 == 0 else rab1
                        with nc.allow_low_precision(reason="softmax recip"):
                            nc.vector.reciprocal(rab[64:65, 0:512], avA[64:65, :])
                            nc.vector.reciprocal(rab[64:65, 512:1024], avB[64:65, :])
                        bcA = psbc.tile([64, 512], F32, tag="bcA")
                        bcB = psbc.tile([64, 512], F32, tag="bcB")
                        nc.tensor.matmul(bcA[:], ones65_sb[:], rab[:, 0:512],
                                         start=True, stop=True)
                        nc.tensor.matmul(bcB[:], ones65_sb[:], rab[:, 512:1024],
                                         start=True, stop=True)
                        bcA_sb = misc.tile([64, 512], F32, tag="bcA_sb")
                        bcB_sb = misc.tile([64, 512], F32, tag="bcB_sb")
                        nc.scalar.copy(bcA_sb[:], bcA[:])
                        nc.scalar.copy(bcB_sb[:], bcB[:])
                        qsl = slice(qc * 512, (qc + 1) * 512)
                        nc.vector.tensor_mul(attnT[0:64, hp, qsl], avA[0:64, :],
                                             bcA_sb[:])
                        stageB = misc.tile([64, 512], F32R, tag="stageB")
                        nc.vector.tensor_mul(stageB[:], avB[0:64, :], bcB_sb[:])
                        nc.sync.dma_start(attnT[64:128, hp, qsl], stageB[:])

            # ---- phase 4: projection ----
            with tc.tile_pool(name="pspj", bufs=2, space="PSUM") as pspj:
                for st in range(NST):
                    out_sb = misc.tile([128, H], F32, tag="out_sb")
                    ssl = slice(st * 128, (st + 1) * 128)
                    for cc in range(2):
                        csl = slice(cc * 384, (cc + 1) * 384)
                        pjA = pspj.tile([128, 384], F32, tag="pjA")
                        pjB = pspj.tile([128, 384], F32, tag="pjB")
                        for t in range(NHT):
                            nc.tensor.matmul(pjA[:], attnT[0:64, t, ssl],
                                             wp_sb[0:64, t, csl],
                                             start=(t == 0), stop=(t == NHT - 1),
                                             tile_position=(0, 0))
                            nc.tensor.matmul(pjB[:], attnT[64:128, t, ssl],
                                             wp_sb[64:128, t, csl],
                                             start=(t == 0), stop=(t == NHT - 1),
                                             tile_position=(64, 0))
                        cpB = misc.tile([128, 384], F32, tag="cpB")
                        nc.scalar.copy(cpB[:], pjB[:])
                        nc.vector.tensor_add(out_sb[:, csl], pjA[:], cpB[:])
                        nc.vector.tensor_add(out_sb[:, csl], out_sb[:, csl],
                                             bpb_sb[:, csl])
                    nc.sync.dma_start(out_d[ssl, :], out_sb[:])

    nc.compile()
    return nc


def make_inputs(x_b, W_qkv, b_qkv, W_proj, b_proj):
    """Host-side constants + per-core input map for one batch element."""
    mask = np.triu(np.ones((128, 128), dtype=np.float32))
    ones65 = np.zeros((65, 64), dtype=np.float32)
    ones65[64, :] = 1.0
    return {
        "x": np.ascontiguousarray(x_b, dtype=np.float32),
        "wqk": np.ascontiguousarray(W_qkv[:, :2 * H], dtype=np.float32),
        "wv": np.ascontiguousarray(W_qkv[:, 2 * H:], dtype=np.float32),
        "wp": np.ascontiguousarray(W_proj, dtype=np.float32),
        "bqk": np.ascontiguousarray(
            b_qkv[:2 * H].reshape(NH, 128).T, dtype=np.float32),
        "bvb": np.broadcast_to(b_qkv[2 * H:], (128, H)).astype(np.float32),
        "bpb": np.broadcast_to(b_proj, (128, H)).astype(np.float32),
        "mask": mask,
        "ones65": ones65,
        "zer64": np.zeros((64, 1024), dtype=np.float32),
        "vones": np.ones((128, NST * NH), dtype=np.float32),
        "ident": np.eye(128, dtype=np.float32),
    }


def get_nc():
    if "nc" not in _CACHE:
        _CACHE["nc"] = build()
    return _CACHE["nc"]


def kernel(x, W_qkv, b_qkv, W_proj, b_proj):
    x = np.asarray(x, dtype=np.float32)
    W_qkv = np.asarray(W_qkv, dtype=np.float32)
    b_qkv = np.asarray(b_qkv, dtype=np.float32)
    W_proj = np.asarray(W_proj, dtype=np.float32)
    b_proj = np.asarray(b_proj, dtype=np.float32)

    nc = get_nc()
    in_maps = [make_inputs(x[b], W_qkv, b_qkv, W_proj, b_proj) for b in range(B)]
    res = run_bass_kernel_spmd(nc, in_maps, list(range(B)))
    return np.stack([res.results[b]["out"] for b in range(B)], axis=0)


# revision 10
# speedup vs baseline: 1.2247x; 1.2247x over previous
"""GPT2 multi-head causal self-attention on 8 trn2 NeuronCores.

Sharding: data-parallel over batch (B=8 -> one batch element per core).
Each core computes its full attention block independently; no collectives.

Per-core dataflow (S=1024, H=768, nh=12, hd=64):
  x [S,H] --PE-transpose--> xT [H,S]                       (fp32 transpose)
  qkT[c,s] = W_qkv[:, :1536].T @ xT    f32r matmuls, bf16 out (ACT +bias)
  v[s,c]   = x @ W_qkv[:, 1536:]       f32r matmuls, bf16 out (DVE +bias),
                                       ones column appended per head
  per head pair (2 heads per 128-partition tile, row-packed matmuls):
    scoresT[k,q] = KT.T @ QT   bf16, causal chunks, A/B -> 2 PSUM banks
    expT = exp(scores/8)       one ACT op per (pair,kt,chunk) covers A+B,
                               bf16 out, no max-sub (|scores/8| < ~3)
    tri-mask on diagonal blocks (DVE, both heads per op)
    att_unT[d,q]+denom = [V_h|1].T @ expT   bf16, PSUM-accum over k
    recip(denom) -> PE broadcast (f32r) -> DVE mult -> attnT [d,q] f32r
    (odd head's 64 rows DMA-shifted to partitions 64..127)
  out = attnT.T @ W_proj + b_proj   f32r row-packed pairs, natural layout
"""
from contextlib import contextmanager

import numpy as np

import concourse.bacc as bacc
import concourse.mybir as mybir
import concourse.tile as tile
from concourse.bass_utils import run_bass_kernel_spmd

F32 = mybir.dt.float32
F32R = mybir.dt.float32r
BF16 = mybir.dt.bfloat16
AF = mybir.ActivationFunctionType

B, S, H = 8, 1024, 768
NH, HD = 12, 64
NHT = H // 128          # 6  h-tiles
NST = S // 128          # 8  s-tiles
NQC = S // 512          # 2  q chunks
NPAIR = NH // 2         # 6  head pairs
SCALE = HD ** -0.5

_CACHE = {}


@contextmanager
def _xpools(tc):
    with (
        tc.tile_pool(name="xph", bufs=1) as xph,
        tc.tile_pool(name="xtp", bufs=1) as xtp,
    ):
        yield xph, xtp


def build(repeat=1, stages=4):
    nc = bacc.Bacc("TRN2", target_bir_lowering=False, debug=False)

    x_d = nc.declare_dram_parameter("x", [S, H], F32, isOutput=False)
    wqk_d = nc.declare_dram_parameter("wqk", [H, 2 * H], F32R, isOutput=False)
    wv_d = nc.declare_dram_parameter("wv", [H, H], F32R, isOutput=False)
    wp_d = nc.declare_dram_parameter("wp", [H, H], BF16, isOutput=False)
    bqk_d = nc.declare_dram_parameter("bqk", [128, NH], F32, isOutput=False)
    bvb_d = nc.declare_dram_parameter("bvb", [128, H], F32, isOutput=False)
    bpb_d = nc.declare_dram_parameter("bpb", [128, H], F32, isOutput=False)
    mask_d = nc.declare_dram_parameter("mask", [128, 256], BF16, isOutput=False)
    ones65_d = nc.declare_dram_parameter("ones65", [65, 64], F32R, isOutput=False)
    zer64_d = nc.declare_dram_parameter("zer64", [64, 1024], F32R, isOutput=False)
    vones_d = nc.declare_dram_parameter("vones", [128, NST * NH], BF16,
                                        isOutput=False)
    ident_d = nc.declare_dram_parameter("ident", [128, 128], F32, isOutput=False)
    out_d = nc.declare_dram_parameter("out", [S, H], F32, isOutput=True)

    with tile.TileContext(nc) as tc:
        with (
            tc.tile_pool(name="perm", bufs=1) as perm,
            tc.tile_pool(name="expp", bufs=2) as expp,
            tc.tile_pool(name="misc", bufs=2) as misc,
        ):
            # ---- persistent SBUF ----
            wp_sb = perm.tile([128, NHT, H], BF16)
            bqk_sb = perm.tile([128, NH], F32)
            bvb_sb = perm.tile([128, H], F32)
            bpb_sb = perm.tile([128, H], F32)
            mask_sb = perm.tile([128, 2, 128], BF16)
            ones65_sb = perm.tile([65, 64], F32R)
            ident_sb = perm.tile([128, 128], F32)
            rab = perm.tile([65, 1024], F32R)
            qkT = perm.tile([128, NH, S], BF16)
            v_sb = perm.tile([128, NST, NH, 65], BF16)
            attnT = perm.tile([128, NHT, S], BF16)

            nc.sync.dma_start(ident_sb[:], ident_d[:])

            for _rep in range(repeat):
                # ---- phase 1: transpose x into xT [h, s] ----
                with _xpools(tc) as (xph, xtp):
                    xT = xph.tile([128, NHT, S], F32R)
                    with tc.tile_pool(name="pst", bufs=4, space="PSUM") as pst:
                        for rnd in range(4):
                            x_tmp = xtp.tile([128, 2, H], F32, tag="x_tmp")
                            nc.sync.dma_start(
                                x_tmp[:],
                                x_d[rnd * 256:(rnd + 1) * 256, :].rearrange(
                                    "(st p) c -> p st c", p=128))
                            for st2 in range(2):
                                st = rnd * 2 + st2
                                for ht in range(NHT):
                                    ps_t = pst.tile([128, 128], F32, tag="ps_t")
                                    nc.tensor.transpose(
                                        ps_t[:],
                                        x_tmp[:, st2, ht * 128:(ht + 1) * 128],
                                        ident_sb[:])
                                    nc.scalar.copy(
                                        xT[:, ht, st * 128:(st + 1) * 128],
                                        ps_t[:])

                    if _rep == 0:
                        nc.sync.dma_start(bqk_sb[:], bqk_d[:])
                        nc.sync.dma_start(bvb_sb[:], bvb_d[:])
                        nc.sync.dma_start(mask_sb[:], mask_d[:].rearrange(
                            "p (a b) -> p a b", a=2))
                        nc.sync.dma_start(ones65_sb[:], ones65_d[:])
                        nc.sync.dma_start(rab[0:64, :], zer64_d[:])
                        nc.sync.dma_start(
                            v_sb[:, :, :, 64],
                            vones_d[:].rearrange("p (a b) -> p a b", a=NST))
                        nc.sync.dma_start(bpb_sb[:], bpb_d[:])
                        for ht in range(NHT):
                            nc.sync.dma_start(
                                wp_sb[:, ht, :], wp_d[ht * 128:(ht + 1) * 128, :])

                    if stages < 2:
                        continue

                    # score PSUM opened early: scores/exp overlap QKV tail
                    with tc.tile_pool(name="pssc", bufs=2, space="PSUM") as pssc:
                        # ---- phase 2a: Q,K transposed (bf16 out) ----
                        with (
                            tc.tile_pool(name="wqkp", bufs=2) as wqkp,
                            tc.tile_pool(name="psqk", bufs=3,
                                         space="PSUM") as psqk,
                        ):
                            for qtr in range(4):
                                wq_q = wqkp.tile([128, NHT, 384], F32R,
                                                 tag="wq_q")
                                for ht in range(NHT):
                                    nc.sync.dma_start(
                                        wq_q[:, ht, :],
                                        wqk_d[ht * 128:(ht + 1) * 128,
                                              qtr * 384:(qtr + 1) * 384])
                                for tl in range(3):
                                    t = qtr * 3 + tl
                                    for qc in range(NQC):
                                        ps = psqk.tile([128, 512], F32, tag="ps")
                                        for ht in range(NHT):
                                            nc.tensor.matmul(
                                                ps[:],
                                                wq_q[:, ht,
                                                     tl * 128:(tl + 1) * 128],
                                                xT[:, ht,
                                                   qc * 512:(qc + 1) * 512],
                                                start=(ht == 0),
                                                stop=(ht == NHT - 1))
                                        nc.scalar.activation(
                                            qkT[:, t, qc * 512:(qc + 1) * 512],
                                            ps[:], AF.Identity,
                                            bias=bqk_sb[:, t:t + 1])

                        # ---- phase 2b: V natural (+bias, bf16 out) ----
                        with (
                            tc.tile_pool(name="wvp", bufs=1) as wvp,
                            tc.tile_pool(name="psv", bufs=3,
                                         space="PSUM") as psv,
                        ):
                            wv_sb = wvp.tile([128, NHT, H], F32R)
                            for ht in range(NHT):
                                nc.sync.dma_start(
                                    wv_sb[:, ht, :],
                                    wv_d[ht * 128:(ht + 1) * 128, :])
                            for st in range(NST):
                                for cc in range(2):
                                    psv_t = psv.tile([128, 384], F32,
                                                     tag="psv_t")
                                    for ht in range(NHT):
                                        nc.tensor.matmul(
                                            psv_t[:],
                                            xT[:, ht, st * 128:(st + 1) * 128],
                                            wv_sb[:, ht,
                                                  cc * 384:(cc + 1) * 384],
                                            start=(ht == 0),
                                            stop=(ht == NHT - 1))
                                    nc.vector.tensor_add(
                                        v_sb[:, st, cc * 6:(cc + 1) * 6, 0:64],
                                        psv_t[:].rearrange(
                                            "p (g d) -> p g d", g=6),
                                        bvb_sb[:, cc * 384:(cc + 1) * 384]
                                        .rearrange("p (g d) -> p g d", g=6))

                        if stages < 3:
                            continue

                        # ---- phase 3: attention per head pair ----
                        with (
                            tc.tile_pool(name="psav", bufs=1,
                                         space="PSUM") as psav,
                            tc.tile_pool(name="psbc", bufs=1,
                                         space="PSUM") as psbc,
                        ):
                            def scores_exp(hp):
                                expAB = expp.tile([128, 2, NST, S], BF16,
                                                  tag="expAB")
                                for kt in range(NST):
                                    qc0 = kt // 4
                                    d = kt * 128 - qc0 * 512
                                    for qc in range(qc0, NQC):
                                        cs = d if qc == qc0 else 0
                                        qsl = slice(qc * 512 + cs,
                                                    (qc + 1) * 512)
                                        ksl = slice(kt * 128, (kt + 1) * 128)
                                        psAB = pssc.tile([128, 2, 512], F32,
                                                         tag="psAB")
                                        nc.tensor.matmul(
                                            psAB[:, 0, cs:512],
                                            qkT[0:64, 6 + hp, ksl],
                                            qkT[0:64, hp, qsl],
                                            start=True, stop=True,
                                            tile_position=(0, 0))
                                        nc.tensor.matmul(
                                            psAB[:, 1, cs:512],
                                            qkT[64:128, 6 + hp, ksl],
                                            qkT[64:128, hp, qsl],
                                            start=True, stop=True,
                                            tile_position=(64, 0))
                                        nc.scalar.activation(
                                            expAB[:, :, kt, qsl],
                                            psAB[:, :, cs:512],
                                            AF.Exp, scale=SCALE)
                                    dsl = slice(kt * 128, kt * 128 + 128)
                                    nc.vector.tensor_mul(
                                        expAB[:, :, kt, dsl],
                                        expAB[:, :, kt, dsl], mask_sb[:])
                                return expAB

                            def attnv(hp, expAB):
                                for qc in range(NQC):
                                    avA = psav.tile([65, 512], F32, tag="avA")
                                    avB = psav.tile([65, 512], F32, tag="avB")
                                    kts = list(range(4 * (qc + 1)))
                                    for kt in kts:
                                        cs = ((kt * 128 - qc * 512)
                                              if kt // 4 == qc else 0)
                                        qsl = slice(qc * 512 + cs,
                                                    (qc + 1) * 512)
                                        st_, sp_ = kt == kts[0], kt == kts[-1]
                                        nc.tensor.matmul(
                                            avA[:, cs:512],
                                            v_sb[:, kt, 2 * hp, :],
                                            expAB[:, 0, kt, qsl],
                                            start=st_, stop=sp_)
                                        nc.tensor.matmul(
                                            avB[:, cs:512],
                                            v_sb[:, kt, 2 * hp + 1, :],
                                            expAB[:, 1, kt, qsl],
                                            start=st_, stop=sp_)
                                    with nc.allow_low_precision(
                                            reason="softmax recip"):
                                        nc.vector.reciprocal(
                                            rab[64:65, 0:512], avA[64:65, :])
                                        nc.vector.reciprocal(
                                            rab[64:65, 512:1024],
                                            avB[64:65, :])
                                    bcA = psbc.tile([64, 512], F32, tag="bcA")
                                    bcB = psbc.tile([64, 512], F32, tag="bcB")
                                    nc.tensor.matmul(bcA[:], ones65_sb[:],
                                                     rab[:, 0:512],
                                                     start=True, stop=True)
                                    nc.tensor.matmul(bcB[:], ones65_sb[:],
                                                     rab[:, 512:1024],
                                                     start=True, stop=True)
                                    bcA_sb = misc.tile([64, 512], F32,
                                                       tag="bcA_sb", bufs=1)
                                    bcB_sb = misc.tile([64, 512], F32,
                                                       tag="bcB_sb", bufs=1)
                                    nc.vector.tensor_copy(bcA_sb[:], bcA[:])
                                    nc.vector.tensor_copy(bcB_sb[:], bcB[:])
                                    qsl = slice(qc * 512, (qc + 1) * 512)
                                    nc.vector.tensor_mul(
                                        attnT[0:64, hp, qsl], avA[0:64, :],
                                        bcA_sb[:])
                                    stageB = misc.tile([64, 512], BF16,
                                                       tag="stageB", bufs=1)
                                    nc.vector.tensor_mul(stageB[:],
                                                         avB[0:64, :],
                                                         bcB_sb[:])
                                    nc.sync.dma_start(
                                        attnT[64:128, hp, qsl], stageB[:])

                            prev = scores_exp(0)
                            for hp in range(1, NPAIR):
                                cur = scores_exp(hp)
                                attnv(hp - 1, prev)
                                prev = cur
                            attnv(NPAIR - 1, prev)

                if stages < 4:
                    continue

                # ---- phase 4: projection ----
                with tc.tile_pool(name="pspj", bufs=2, space="PSUM") as pspj:
                    for st in range(NST):
                        out_sb = misc.tile([128, H], F32, tag="out_sb")
                        ssl = slice(st * 128, (st + 1) * 128)
                        for cc in range(2):
                            csl = slice(cc * 384, (cc + 1) * 384)
                            pjA = pspj.tile([128, 384], F32, tag="pjA")
                            for t in range(NHT):
                                nc.tensor.matmul(
                                    pjA[:], attnT[:, t, ssl], wp_sb[:, t, csl],
                                    start=(t == 0), stop=(t == NHT - 1))
                            nc.vector.tensor_add(out_sb[:, csl], pjA[:],
                                                 bpb_sb[:, csl])
                        nc.sync.dma_start(out_d[ssl, :], out_sb[:])

    nc.compile()
    return nc


def make_inputs(x_b, W_qkv, b_qkv, W_proj, b_proj):
    """Host-side constants + per-core input map for one batch element."""
    import ml_dtypes
    mask = np.tile(np.triu(np.ones((128, 128), dtype=np.float32)), (1, 2))
    ones65 = np.zeros((65, 64), dtype=np.float32)
    ones65[64, :] = 1.0
    return {
        "x": np.ascontiguousarray(x_b, dtype=np.float32),
        "wqk": np.ascontiguousarray(W_qkv[:, :2 * H], dtype=np.float32),
        "wv": np.ascontiguousarray(W_qkv[:, 2 * H:], dtype=np.float32),
        "wp": np.ascontiguousarray(W_proj).astype(ml_dtypes.bfloat16),
        "bqk": np.ascontiguousarray(
            b_qkv[:2 * H].reshape(NH, 128).T, dtype=np.float32),
        "bvb": np.broadcast_to(b_qkv[2 * H:], (128, H)).astype(np.float32),
        "bpb": np.broadcast_to(b_proj, (128, H)).astype(np.float32),
        "mask": mask.astype(ml_dtypes.bfloat16),
        "ones65": ones65,
        "zer64": np.zeros((64, 1024), dtype=np.float32),
        "vones": np.ones((128, NST * NH), dtype=ml_dtypes.bfloat16),
        "ident": np.eye(128, dtype=np.float32),
    }


def get_nc(repeat=1, stages=4):
    key = f"nc{repeat}s{stages}"
    if key not in _CACHE:
        _CACHE[key] = build(repeat, stages)
    return _CACHE[key]


def kernel(x, W_qkv, b_qkv, W_proj, b_proj):
    x = np.asarray(x, dtype=np.float32)
    W_qkv = np.asarray(W_qkv, dtype=np.float32)
    b_qkv = np.asarray(b_qkv, dtype=np.float32)
    W_proj = np.asarray(W_proj, dtype=np.float32)
    b_proj = np.asarray(b_proj, dtype=np.float32)

    nc = get_nc()
    in_maps = [make_inputs(x[b], W_qkv, b_qkv, W_proj, b_proj) for b in range(B)]
    res = run_bass_kernel_spmd(nc, in_maps, list(range(B)))
    return np.stack([res.results[b]["out"] for b in range(B)], axis=0)
